# revision 21
# baseline (speedup 1.0000x reference)
"""BitStackLinear Trainium2 kernel.

Computes out = x @ w.T where w = sum_i sign_i * (u_i @ vt_i), signs unpacked
from 4 packed bit-planes (one byte = 8 signs, little-endian).

Tensor-parallel over out_features across 8 NeuronCores (1376 cols each).
Per core, everything in fp16 on the PE (1 moving col/cycle @2.4GHz; rel err
~5.6e-4 vs the 2e-2 gate):

  Prep: vts = vt * 2^(1-(k%8)) cast to fp16 (via DRAM roundtrip), u.T fp16.
  Recon of w.T into SBUF, fp16, per 128-row k-slab, per o-half [688 | 688]:
    PE:  r = vts_i.T @ u16_i  -> PSUM (rank-16 fp16 matmuls)
    DVE: a = bytes & bitmask (i32, on pre-replicated qbE bytes),
         t = (a - 2^(j-1)) * r (STT reading PSUM, fp16 out; the 2^(1-j)
         descale is pre-folded into vts), acc via ADD tree (one ADD on
         GpSimd to unload DVE, the prologue pacer).
  GEMM (two passes over the o-halves; pass 1 overlaps recon of half 2):
    stationary = x.T tile [128k, 128m] fp16 (2MB f32 gather DMA from the
    host-pre-tiled x5d layout + DVE cast; issue alternates SP/Activation
    DGE queues), moving = w.T slab [128k, 512|176] fp16 from SBUF,
    PSUM [128m, 688] accumulated over 32 k-slabs, ScalarE evac to fp16,
    output DMA batched 2 mtiles into per-pass contiguous DRAM (single big
    descriptors; 128-row strided writes cost ~4.4us each otherwise).

kernel(**inputs) takes the full unsharded inputs, returns the full output.
Host work is layout only: transposes, byte replication, dtype reinterpret,
sharding. All value-changing math (pps scaling, fp16 rounding, bit unpack,
sign apply, GEMM) runs on device.
"""

import numpy as np

import concourse.bass as bass
import concourse.bacc as bacc
import concourse.mybir as mybir
import concourse.tile as tile

W_BIT = 4
OUT_F = 11008
IN_F = 4096
RANK = 16
NCORES = 8
O_SHARD = OUT_F // NCORES          # 1376
K_TILES = IN_F // 128              # 32
OH1 = 688                          # o-half 1 (GEMM pass 1)
OH2 = O_SHARD - OH1                # 688
HCH = [(0, 512), (512, 176)]       # per-half psum chunks


def _body(tc, aps, M):
    nc = tc.nc
    x5d, qbE1, qbE2, uT, vt, pps, bm, hm, vts_d, out1, out2 = (
        aps["x5d"], aps["qbE1"], aps["qbE2"], aps["uT"], aps["vt"],
        aps["pps"], aps["bm"], aps["hm"], aps["vts_d"],
        aps["out1"], aps["out2"])
    f32, f16, u8, i32 = (mybir.dt.float32, mybir.dt.float16, mybir.dt.uint8,
                         mybir.dt.int32)
    NMT = M // 128

    import contextlib
    with contextlib.ExitStack() as ctx:
        pool = ctx.enter_context(tc.tile_pool(name="sb", bufs=1))
        psum = ctx.enter_context(tc.tile_pool(name="ps", bufs=1, space="PSUM"))

        # ---- constants ----
        bm_t = pool.tile([128, W_BIT, O_SHARD], u8, name="bm_t")
        nc.sync.dma_start(bm_t, bm)
        bm32 = bm_t.bitcast(i32)
        hm_t = pool.tile([128, 1], f32, name="hm_t")
        nc.sync.dma_start(hm_t, hm)

        # ---- vts_d = vt * 2^(1-(k%8)) as fp16, via DRAM ----
        pps_t = pool.tile([16, 512], f32, name="pps_t")
        nc.scalar.dma_start(pps_t, pps)
        for c in range(8):
            for i in range(W_BIT):
                vstg = pool.tile([16, 512], f32, name=f"vstg{i}_{c}",
                                 tag="vstg", bufs=2)
                nc.scalar.dma_start(vstg, vt[i, :, c * 512:(c + 1) * 512])
                vs16 = pool.tile([16, 512], f16, name=f"vs16{i}_{c}",
                                 tag="vs16", bufs=2)
                nc.vector.tensor_tensor(out=vs16, in0=vstg, in1=pps_t,
                                        op=mybir.AluOpType.mult)
                nc.scalar.dma_start(vts_d[i, :, c * 512:(c + 1) * 512], vs16)
        # ---- ut16 resident [16, 4, O_SHARD] fp16 ----
        ut16 = pool.tile([16, W_BIT, O_SHARD], f16, name="ut16")
        for i in range(W_BIT):
            for c, (c0, cw) in enumerate([(0, 688), (688, 688)]):
                ustg = pool.tile([16, 688], f32, name=f"ustg{i}_{c}",
                                 tag="ustg", bufs=2)
                nc.scalar.dma_start(ustg, uT[i, :, c0:c0 + cw])
                nc.vector.tensor_copy(ut16[:, i, c0:c0 + cw], ustg)

        # ---- persistent w.T halves ----
        w1 = [pool.tile([128, OH1], f16, name=f"w1_{ks}", tag="w1",
                        bufs=K_TILES) for ks in range(K_TILES)]
        w2 = [pool.tile([128, OH2], f16, name=f"w2_{ks}", tag="w2",
                        bufs=K_TILES) for ks in range(K_TILES)]

        def recon(ks, half):
            o0, ow = (0, OH1) if half == 1 else (OH1, OH2)
            wt = w1[ks] if half == 1 else w2[ks]
            chunks = HCH
            vtb = pool.tile([16, W_BIT, 128], f16, name=f"vtb{ks}_{half}",
                            tag="vtb", bufs=3)
            src = vts_d.rearrange("i r k -> r i k")
            nc.sync.dma_start(vtb, src[:, :, ks * 128:(ks + 1) * 128])
            bts = pool.tile([128, W_BIT, ow], u8, name=f"bts{ks}_{half}",
                            tag=f"bts{half}", bufs=2)
            nc.sync.dma_start(bts, (qbE1 if half == 1 else qbE2)[ks])
            a4 = bts
            nc.vector.tensor_tensor(out=a4.bitcast(i32),
                                    in0=bts.bitcast(i32),
                                    in1=bm32[:, :, o0 // 4:(o0 + ow) // 4],
                                    op=mybir.AluOpType.bitwise_and)
            ts16 = [None] * W_BIT
            for i in range(W_BIT):
                t16 = None
                if i > 0:
                    t16 = pool.tile([128, ow], f16, name=f"t{ks}_{half}_{i}",
                                    tag=f"t{half}", bufs=4)
                    ts16[i] = t16
                for (c0, cw) in chunks:
                    pr = psum.tile([128, cw], f32, name=f"pr{ks}_{half}_{i}_{c0}",
                                   tag=f"rp{cw}", bufs=2)
                    nc.tensor.matmul(pr, vtb[:, i, :],
                                     ut16[:, i, o0 + c0:o0 + c0 + cw],
                                     start=True, stop=True)
                    dst = wt if i == 0 else t16
                    nc.vector.scalar_tensor_tensor(
                        out=dst[:, c0:c0 + cw], in0=a4[:, i, c0:c0 + cw],
                        scalar=hm_t, in1=pr,
                        op0=mybir.AluOpType.subtract, op1=mybir.AluOpType.mult)
                if i == 1:
                    # accumulate tree entirely on gpsimd: DVE (the prologue
                    # pacer) keeps only AND + the 4 STTs
                    nc.gpsimd.tensor_tensor(out=wt, in0=wt, in1=t16,
                                            op=mybir.AluOpType.add)
                if i == 3:
                    nc.gpsimd.tensor_tensor(out=ts16[2], in0=ts16[2],
                                            in1=ts16[3],
                                            op=mybir.AluOpType.add)
                    nc.gpsimd.tensor_tensor(out=wt, in0=wt, in1=ts16[2],
                                            op=mybir.AluOpType.add)

        # ---- recon half 1 + PE warm-keeping filler ----
        for ks in range(K_TILES):
            recon(ks, 1)
            for j in range(3):
                jp = psum.tile([128, 512], f32, name=f"jk{ks}_{j}",
                               tag="pg512", bufs=2)
                nc.tensor.matmul(jp, ut16[:, 0, 0:128], ut16[:, 0, 0:512],
                                 start=True, stop=True)

        # ---- GEMM: triangular schedule ----
        # A: mt 0..31 x half1 (recon h2 overlapped); B: mt 32..63 x FULL
        # width (x read once, 4 mms/stationary, psum reuses retired recon
        # banks); C: mt 0..31 x half2.
        x8 = [None] * NMT
        x16 = [None] * NMT

        def dma_x(tag_sfx, mt):
            t = pool.tile([128, K_TILES, 128], f32,
                          name=f"x8_{tag_sfx}_{mt}", tag="x8", bufs=2)
            eng = nc.sync if mt % 2 == 0 else nc.scalar
            eng.dma_start(t, x5d[mt])
            x8[mt] = t

        def cast_x(tag_sfx, mt):
            t = pool.tile([128, K_TILES, 128], f16,
                          name=f"x16_{tag_sfx}_{mt}", tag="x16", bufs=3)
            nc.vector.tensor_copy(
                t.rearrange("p k m -> p (k m)"),
                x8[mt].rearrange("p k m -> p (k m)"))
            x16[mt] = t

        def gemm_pass(name, mts, targets, interleave_recon):
            # targets: list of (wlist, outd, o_off, psum_tag_prefix)
            osts = {}
            dma_x(name, mts[0])
            dma_x(name, mts[1])
            cast_x(name, mts[0])
            for j, mt in enumerate(mts):
                if j + 2 < len(mts):
                    dma_x(name, mts[j + 2])
                if j + 1 < len(mts):
                    cast_x(name, mts[j + 1])
                pgs = []
                for (wlist, outd, o_off, ptag) in targets:
                    for (c0, cw) in HCH:
                        pg = psum.tile([128, cw], f32,
                                       name=f"pg{name}_{mt}_{o_off}_{c0}",
                                       tag=f"{ptag}{cw}", bufs=2)
                        pgs.append((pg, wlist, c0, cw))
                for ks in range(K_TILES):
                    for (pg, wlist, c0, cw) in pgs:
                        nc.tensor.matmul(pg, x16[mt][:, ks, :],
                                         wlist[ks][:, c0:c0 + cw],
                                         start=(ks == 0),
                                         stop=(ks == K_TILES - 1))
                for ti, (wlist, outd, o_off, ptag) in enumerate(targets):
                    if j % 2 == 0:
                        osts[ti] = pool.tile([128, 2, 688], f16,
                                             name=f"ost{name}_{mt}_{o_off}",
                                             tag=f"ost{o_off}", bufs=2)
                    for (pg, wl, c0, cw) in pgs:
                        if wl is wlist:
                            nc.scalar.copy(osts[ti][:, j % 2, c0:c0 + cw], pg)
                    if j % 2 == 1:
                        dst = outd.rearrange("(t p) o -> p t o", p=128)
                        nc.scalar.dma_start(dst[:, mt - 1:mt + 1, :],
                                            osts[ti])
                if interleave_recon and j < K_TILES:
                    recon(j, 2)

        gemm_pass("A", list(range(0, K_TILES)), [(w1, out1, 0, "pg")], True)
        gemm_pass("B", list(range(K_TILES, NMT)),
                  [(w1, out1, 0, "pg"), (w2, out2, 688, "rp")], False)
        gemm_pass("C", list(range(0, K_TILES)), [(w2, out2, 688, "pg")],
                  False)


def build_bass(M=8192):
    nc = bacc.Bacc("TRN2", target_bir_lowering=False, debug=False)
    f32, f16, u8 = mybir.dt.float32, mybir.dt.float16, mybir.dt.uint8
    aps = {}
    aps["x5d"] = nc.dram_tensor("x5d", [M // 128, 128, K_TILES, 128], f32,
                                kind="ExternalInput").ap()
    aps["qbE1"] = nc.dram_tensor("qbE1", [K_TILES, 128, W_BIT, OH1], u8,
                                 kind="ExternalInput").ap()
    aps["qbE2"] = nc.dram_tensor("qbE2", [K_TILES, 128, W_BIT, OH2], u8,
                                 kind="ExternalInput").ap()
    aps["uT"] = nc.dram_tensor("uT", [W_BIT, RANK, O_SHARD], f32,
                               kind="ExternalInput").ap()
    aps["vt"] = nc.dram_tensor("vt", [W_BIT, RANK, IN_F], f32,
                               kind="ExternalInput").ap()
    aps["pps"] = nc.dram_tensor("pps", [16, 512], f32,
                                kind="ExternalInput").ap()
    aps["bm"] = nc.dram_tensor("bm", [128, W_BIT, O_SHARD], u8,
                               kind="ExternalInput").ap()
    aps["hm"] = nc.dram_tensor("hm", [128, 1], f32, kind="ExternalInput").ap()
    aps["vts_d"] = nc.dram_tensor("vts_d", [W_BIT, RANK, IN_F], f16,
                                  kind="Internal").ap()
    aps["out1"] = nc.dram_tensor("out1", [M, OH1], f16,
                                 kind="ExternalOutput").ap()
    aps["out2"] = nc.dram_tensor("out2", [M, OH2], f16,
                                 kind="ExternalOutput").ap()
    with tile.TileContext(nc) as tc:
        _body(tc, aps, M)
    nc.compile()
    return nc


def prep_inputs(x, qweight, u, vt):
    """Host-side layout prep (transposes / dtype views / sharding only)."""
    M = x.shape[0] * x.shape[1]
    # x5d[mt, p, k, m] = x[mt*128 + m, k*128 + p] (pure transpose/tiling)
    x5d = np.ascontiguousarray(
        x.reshape(M // 128, 128, K_TILES, 128).transpose(0, 3, 2, 1))
    qb = qweight.astype(np.uint8)  # values 0..255 stored in int32
    p = np.arange(128)
    bm = (np.uint8(1) << (p % 8).astype(np.uint8))[:, None, None] * np.ones(
        (1, W_BIT, O_SHARD), np.uint8)
    hm = (2.0 ** ((p % 8) - 1.0)).astype(np.float32).reshape(128, 1)
    pps = np.tile((2.0 ** (1.0 - (np.arange(512) % 8))).astype(np.float32),
                  (16, 1))
    vt_c = np.ascontiguousarray(vt)
    in_maps = []
    for c in range(NCORES):
        sl = slice(c * O_SHARD, (c + 1) * O_SHARD)
        qbT = qb.reshape(W_BIT, OUT_F, IN_F // 8)[:, sl, :].transpose(0, 2, 1)
        # expand to [K_TILES, 128, W_BIT, O_SHARD]: byte replicated 8x along
        # partitions (pure layout: repeat + transpose), split by o-half so
        # each DMA reads per-partition-contiguous runs
        qbE = np.repeat(qbT.reshape(W_BIT, K_TILES, 16, O_SHARD), 8,
                        axis=2).transpose(1, 2, 0, 3)
        qbE1 = np.ascontiguousarray(qbE[:, :, :, :OH1])
        qbE2 = np.ascontiguousarray(qbE[:, :, :, OH1:])
        uT = np.ascontiguousarray(u[:, sl, :].transpose(0, 2, 1))
        in_maps.append({
            "x5d": x5d, "qbE1": qbE1, "qbE2": qbE2, "uT": uT, "vt": vt_c,
            "pps": pps, "bm": bm, "hm": hm,
        })
    return in_maps


def assemble(results, M):
    out = np.empty((M, OUT_F), np.float32)
    for c in range(NCORES):
        out[:, c * O_SHARD:c * O_SHARD + OH1] = results[c]["out1"]
        out[:, c * O_SHARD + OH1:(c + 1) * O_SHARD] = results[c]["out2"]
    return out


def _enable_ldw_opt():
    """No-op: fp16 LDWEIGHTS (~53ns, FWL) fully overlaps matmuls via the PE
    reorder window; walrus ldw-opt is both unnecessary and incompatible with
    the fp16 ldweights this kernel emits."""


def kernel(x, qweight, u, vt):
    from concourse import bass_utils
    _enable_ldw_opt()
    x = np.asarray(x)
    qweight = np.asarray(qweight)
    u = np.asarray(u)
    vt = np.asarray(vt)
    B, S, _ = x.shape
    M = B * S
    nc = build_bass(M)
    in_maps = prep_inputs(x, qweight, u, vt)
    res = bass_utils.run_bass_kernel_spmd(nc, in_maps,
                                          core_ids=list(range(NCORES)))
    return assemble(res.results, M).reshape(B, S, OUT_F)


if __name__ == "__main__":
    rng = np.random.default_rng(0)
    x = rng.standard_normal((4, 2048, IN_F)).astype(np.float32)
    qw = rng.integers(0, 256, size=(W_BIT, OUT_F * IN_F // 8)).astype(np.int32)
    uu = (rng.standard_normal((W_BIT, OUT_F, RANK)) * 0.05).astype(np.float32)
    vv = (rng.standard_normal((W_BIT, RANK, IN_F)) * 0.05).astype(np.float32)
    out = kernel(x=x, qweight=qw, u=uu, vt=vv)
    print(out.shape, out.dtype)


# revision 23
# speedup vs baseline: 1.0738x; 1.0738x over previous
"""BitStackLinear Trainium2 kernel.

Computes out = x @ w.T where w = sum_i sign_i * (u_i @ vt_i), signs unpacked
from 4 packed bit-planes (one byte = 8 signs, little-endian).

Tensor-parallel over out_features across 8 NeuronCores (1376 cols each).
Per core, everything in fp16 on the PE (1 moving col/cycle @2.4GHz; rel err
~5.6e-4 vs the 2e-2 gate):

  Prep: vts = vt * 2^(1-(k%8)) cast to fp16 (via DRAM roundtrip), u.T fp16.
  Recon of w.T into SBUF, fp16, per 128-row k-slab, per o-half [688 | 688]:
    PE:  r = vts_i.T @ u16_i  -> PSUM (rank-16 fp16 matmuls)
    DVE: a = bytes & bitmask (i32, on pre-replicated qbE bytes),
         t = (a - 2^(j-1)) * r (STT reading PSUM, fp16 out; the 2^(1-j)
         descale is pre-folded into vts), acc via ADD tree (one ADD on
         GpSimd to unload DVE, the prologue pacer).
  GEMM (two passes over the o-halves; pass 1 overlaps recon of half 2):
    stationary = x.T tile [128k, 128m] fp16 (2MB f32 gather DMA from the
    host-pre-tiled x5d layout + DVE cast; issue alternates SP/Activation
    DGE queues), moving = w.T slab [128k, 512|176] fp16 from SBUF,
    PSUM [128m, 688] accumulated over 32 k-slabs, ScalarE evac to fp16,
    output DMA batched 2 mtiles into per-pass contiguous DRAM (single big
    descriptors; 128-row strided writes cost ~4.4us each otherwise).

kernel(**inputs) takes the full unsharded inputs, returns the full output.
Host work is layout only: transposes, byte replication, dtype reinterpret,
sharding. All value-changing math (pps scaling, fp16 rounding, bit unpack,
sign apply, GEMM) runs on device.
"""

import numpy as np

import concourse.bass as bass
import concourse.bacc as bacc
import concourse.mybir as mybir
import concourse.tile as tile

W_BIT = 4
OUT_F = 11008
IN_F = 4096
RANK = 16
NCORES = 8
O_SHARD = OUT_F // NCORES          # 1376
K_TILES = IN_F // 128              # 32
OH1 = 688                          # o-half 1 (GEMM pass 1)
OH2 = O_SHARD - OH1                # 688
HCH = [(0, 512), (512, 176)]       # per-half psum chunks


def _body(tc, aps, M):
    nc = tc.nc
    x5d, qbE1, qbE2, uT, vt, pps, bm, hm, vts_d, out1, out2 = (
        aps["x5d"], aps["qbE1"], aps["qbE2"], aps["uT"], aps["vt"],
        aps["pps"], aps["bm"], aps["hm"], aps["vts_d"],
        aps["out1"], aps["out2"])
    f32, f16, u8, i32 = (mybir.dt.float32, mybir.dt.float16, mybir.dt.uint8,
                         mybir.dt.int32)
    NMT = M // 128

    import contextlib
    with contextlib.ExitStack() as ctx:
        pool = ctx.enter_context(tc.tile_pool(name="sb", bufs=1))
        psum = ctx.enter_context(tc.tile_pool(name="ps", bufs=1, space="PSUM"))

        # ---- constants ----
        bm_t = pool.tile([128, W_BIT, O_SHARD], u8, name="bm_t")
        nc.sync.dma_start(bm_t, bm)
        bm32 = bm_t.bitcast(i32)
        hm_t = pool.tile([128, 1], f32, name="hm_t")
        nc.sync.dma_start(hm_t, hm)

        # ---- vts_d = vt * 2^(1-(k%8)) as fp16, via DRAM ----
        pps_t = pool.tile([16, 512], f32, name="pps_t")
        nc.scalar.dma_start(pps_t, pps)
        for c in range(8):
            for i in range(W_BIT):
                vstg = pool.tile([16, 512], f32, name=f"vstg{i}_{c}",
                                 tag="vstg", bufs=2)
                nc.scalar.dma_start(vstg, vt[i, :, c * 512:(c + 1) * 512])
                vs16 = pool.tile([16, 512], f16, name=f"vs16{i}_{c}",
                                 tag="vs16", bufs=2)
                nc.vector.tensor_tensor(out=vs16, in0=vstg, in1=pps_t,
                                        op=mybir.AluOpType.mult)
                nc.scalar.dma_start(vts_d[i, :, c * 512:(c + 1) * 512], vs16)
        # ---- ut16 resident [16, 4, O_SHARD] fp16 ----
        ut16 = pool.tile([16, W_BIT, O_SHARD], f16, name="ut16")
        for i in range(W_BIT):
            for c, (c0, cw) in enumerate([(0, 688), (688, 688)]):
                ustg = pool.tile([16, 688], f32, name=f"ustg{i}_{c}",
                                 tag="ustg", bufs=2)
                nc.scalar.dma_start(ustg, uT[i, :, c0:c0 + cw])
                nc.vector.tensor_copy(ut16[:, i, c0:c0 + cw], ustg)

        # ---- persistent w.T halves ----
        w1 = [pool.tile([128, OH1], f16, name=f"w1_{ks}", tag="w1",
                        bufs=K_TILES) for ks in range(K_TILES)]
        w2 = [pool.tile([128, OH2], f16, name=f"w2_{ks}", tag="w2",
                        bufs=K_TILES) for ks in range(K_TILES)]

        def recon(ks, half):
            o0, ow = (0, OH1) if half == 1 else (OH1, OH2)
            wt = w1[ks] if half == 1 else w2[ks]
            chunks = HCH
            vtb = pool.tile([16, W_BIT, 128], f16, name=f"vtb{ks}_{half}",
                            tag="vtb", bufs=3)
            src = vts_d.rearrange("i r k -> r i k")
            nc.sync.dma_start(vtb, src[:, :, ks * 128:(ks + 1) * 128])
            bts = pool.tile([128, W_BIT, ow], u8, name=f"bts{ks}_{half}",
                            tag=f"bts{half}", bufs=2)
            nc.sync.dma_start(bts, (qbE1 if half == 1 else qbE2)[ks])
            a4 = bts
            nc.vector.tensor_tensor(out=a4.bitcast(i32),
                                    in0=bts.bitcast(i32),
                                    in1=bm32[:, :, o0 // 4:(o0 + ow) // 4],
                                    op=mybir.AluOpType.bitwise_and)
            ts16 = [None] * W_BIT
            for i in range(W_BIT):
                t16 = None
                if i > 0:
                    t16 = pool.tile([128, ow], f16, name=f"t{ks}_{half}_{i}",
                                    tag=f"t{half}", bufs=4)
                    ts16[i] = t16
                for (c0, cw) in chunks:
                    pr = psum.tile([128, cw], f32, name=f"pr{ks}_{half}_{i}_{c0}",
                                   tag=f"rp{cw}", bufs=2)
                    nc.tensor.matmul(pr, vtb[:, i, :],
                                     ut16[:, i, o0 + c0:o0 + c0 + cw],
                                     start=True, stop=True)
                    dst = wt if i == 0 else t16
                    nc.vector.scalar_tensor_tensor(
                        out=dst[:, c0:c0 + cw], in0=a4[:, i, c0:c0 + cw],
                        scalar=hm_t, in1=pr,
                        op0=mybir.AluOpType.subtract, op1=mybir.AluOpType.mult)
                if i == 1:
                    # accumulate tree entirely on gpsimd: DVE (the prologue
                    # pacer) keeps only AND + the 4 STTs
                    nc.gpsimd.tensor_tensor(out=wt, in0=wt, in1=t16,
                                            op=mybir.AluOpType.add)
                if i == 3:
                    nc.gpsimd.tensor_tensor(out=ts16[2], in0=ts16[2],
                                            in1=ts16[3],
                                            op=mybir.AluOpType.add)
                    nc.gpsimd.tensor_tensor(out=wt, in0=wt, in1=ts16[2],
                                            op=mybir.AluOpType.add)

        # ---- recon half 1 + PE warm-keeping filler ----
        for ks in range(K_TILES):
            recon(ks, 1)
            for j in range(3):
                jp = psum.tile([128, 512], f32, name=f"jk{ks}_{j}",
                               tag="pg512", bufs=2)
                nc.tensor.matmul(jp, ut16[:, 0, 0:128], ut16[:, 0, 0:512],
                                 start=True, stop=True)

        # ---- GEMM passes ----
        def gemm_pass(o0, ow, chunks, wlist, interleave_recon, outd):
            first_pass = interleave_recon
            x8 = [None] * NMT
            x16 = [None] * NMT

            def dma_x(mt):
                t = pool.tile([128, K_TILES, 128], f32,
                              name=f"x8_{o0}_{mt}", tag="x8", bufs=2)
                eng = nc.sync if mt % 2 == 0 else nc.scalar
                eng.dma_start(t, x5d[mt])
                x8[mt] = t

            def cast_x(mt):
                t = pool.tile([128, K_TILES, 128], f16, name=f"x16_{o0}_{mt}",
                              tag="x16", bufs=3)
                nc.vector.tensor_copy(
                    t.rearrange("p k m -> p (k m)"),
                    x8[mt].rearrange("p k m -> p (k m)"))
                x16[mt] = t

            ost_cur = [None]
            dma_x(0)
            dma_x(1)
            cast_x(0)
            for mt in range(NMT):
                if mt + 2 < NMT:
                    dma_x(mt + 2)
                if mt + 1 < NMT:
                    cast_x(mt + 1)
                pgs = []
                for (c0, cw) in chunks:
                    # during the recon-h1 drain (first 8 mtiles of pass 1,
                    # before the h2 interleave claims them) borrow the idle
                    # recon psum banks: 4 open groups instead of 2
                    fam = ("rp" if (interleave_recon and mt < 8
                                    and mt % 2 == 1) else "pg")
                    pg = psum.tile([128, cw], f32, name=f"pg{o0}_{mt}_{c0}",
                                   tag=f"{fam}{cw}", bufs=2)
                    pgs.append((pg, c0, cw))
                for ks in range(K_TILES):
                    for (pg, c0, cw) in pgs:
                        nc.tensor.matmul(pg, x16[mt][:, ks, :],
                                         wlist[ks][:, c0:c0 + cw],
                                         start=(ks == 0),
                                         stop=(ks == K_TILES - 1))
                if mt % 2 == 0:
                    ost = pool.tile([128, 2, ow], f16, name=f"ost{o0}_{mt}",
                                    tag=f"ost{o0}", bufs=2)
                    ost_cur[0] = ost
                for (pg, c0, cw) in pgs:
                    nc.scalar.copy(ost_cur[0][:, mt % 2, c0:c0 + cw], pg)
                if mt % 2 == 1:
                    dst = outd.rearrange("(t p) o -> p t o", p=128)
                    nc.scalar.dma_start(dst[:, mt - 1:mt + 1, :], ost_cur[0])
                if interleave_recon and 8 <= mt < 8 + K_TILES:
                    recon(mt - 8, 2)

        gemm_pass(0, OH1, HCH, w1, True, out1)
        gemm_pass(OH1, OH2, HCH, w2, False, out2)


def build_bass(M=8192):
    nc = bacc.Bacc("TRN2", target_bir_lowering=False, debug=False)
    f32, f16, u8 = mybir.dt.float32, mybir.dt.float16, mybir.dt.uint8
    aps = {}
    aps["x5d"] = nc.dram_tensor("x5d", [M // 128, 128, K_TILES, 128], f32,
                                kind="ExternalInput").ap()
    aps["qbE1"] = nc.dram_tensor("qbE1", [K_TILES, 128, W_BIT, OH1], u8,
                                 kind="ExternalInput").ap()
    aps["qbE2"] = nc.dram_tensor("qbE2", [K_TILES, 128, W_BIT, OH2], u8,
                                 kind="ExternalInput").ap()
    aps["uT"] = nc.dram_tensor("uT", [W_BIT, RANK, O_SHARD], f32,
                               kind="ExternalInput").ap()
    aps["vt"] = nc.dram_tensor("vt", [W_BIT, RANK, IN_F], f32,
                               kind="ExternalInput").ap()
    aps["pps"] = nc.dram_tensor("pps", [16, 512], f32,
                                kind="ExternalInput").ap()
    aps["bm"] = nc.dram_tensor("bm", [128, W_BIT, O_SHARD], u8,
                               kind="ExternalInput").ap()
    aps["hm"] = nc.dram_tensor("hm", [128, 1], f32, kind="ExternalInput").ap()
    aps["vts_d"] = nc.dram_tensor("vts_d", [W_BIT, RANK, IN_F], f16,
                                  kind="Internal").ap()
    aps["out1"] = nc.dram_tensor("out1", [M, OH1], f16,
                                 kind="ExternalOutput").ap()
    aps["out2"] = nc.dram_tensor("out2", [M, OH2], f16,
                                 kind="ExternalOutput").ap()
    with tile.TileContext(nc) as tc:
        _body(tc, aps, M)
    nc.compile()
    return nc


def prep_inputs(x, qweight, u, vt):
    """Host-side layout prep (transposes / dtype views / sharding only)."""
    M = x.shape[0] * x.shape[1]
    # x5d[mt, p, k, m] = x[mt*128 + m, k*128 + p] (pure transpose/tiling)
    x5d = np.ascontiguousarray(
        x.reshape(M // 128, 128, K_TILES, 128).transpose(0, 3, 2, 1))
    qb = qweight.astype(np.uint8)  # values 0..255 stored in int32
    p = np.arange(128)
    bm = (np.uint8(1) << (p % 8).astype(np.uint8))[:, None, None] * np.ones(
        (1, W_BIT, O_SHARD), np.uint8)
    hm = (2.0 ** ((p % 8) - 1.0)).astype(np.float32).reshape(128, 1)
    pps = np.tile((2.0 ** (1.0 - (np.arange(512) % 8))).astype(np.float32),
                  (16, 1))
    vt_c = np.ascontiguousarray(vt)
    in_maps = []
    for c in range(NCORES):
        sl = slice(c * O_SHARD, (c + 1) * O_SHARD)
        qbT = qb.reshape(W_BIT, OUT_F, IN_F // 8)[:, sl, :].transpose(0, 2, 1)
        # expand to [K_TILES, 128, W_BIT, O_SHARD]: byte replicated 8x along
        # partitions (pure layout: repeat + transpose), split by o-half so
        # each DMA reads per-partition-contiguous runs
        qbE = np.repeat(qbT.reshape(W_BIT, K_TILES, 16, O_SHARD), 8,
                        axis=2).transpose(1, 2, 0, 3)
        qbE1 = np.ascontiguousarray(qbE[:, :, :, :OH1])
        qbE2 = np.ascontiguousarray(qbE[:, :, :, OH1:])
        uT = np.ascontiguousarray(u[:, sl, :].transpose(0, 2, 1))
        in_maps.append({
            "x5d": x5d, "qbE1": qbE1, "qbE2": qbE2, "uT": uT, "vt": vt_c,
            "pps": pps, "bm": bm, "hm": hm,
        })
    return in_maps


def assemble(results, M):
    out = np.empty((M, OUT_F), np.float32)
    for c in range(NCORES):
        out[:, c * O_SHARD:c * O_SHARD + OH1] = results[c]["out1"]
        out[:, c * O_SHARD + OH1:(c + 1) * O_SHARD] = results[c]["out2"]
    return out


def _enable_ldw_opt():
    """No-op: fp16 LDWEIGHTS (~53ns, FWL) fully overlaps matmuls via the PE
    reorder window; walrus ldw-opt is both unnecessary and incompatible with
    the fp16 ldweights this kernel emits."""


def kernel(x, qweight, u, vt):
    from concourse import bass_utils
    _enable_ldw_opt()
    x = np.asarray(x)
    qweight = np.asarray(qweight)
    u = np.asarray(u)
    vt = np.asarray(vt)
    B, S, _ = x.shape
    M = B * S
    nc = build_bass(M)
    in_maps = prep_inputs(x, qweight, u, vt)
    res = bass_utils.run_bass_kernel_spmd(nc, in_maps,
                                          core_ids=list(range(NCORES)))
    return assemble(res.results, M).reshape(B, S, OUT_F)


if __name__ == "__main__":
    rng = np.random.default_rng(0)
    x = rng.standard_normal((4, 2048, IN_F)).astype(np.float32)
    qw = rng.integers(0, 256, size=(W_BIT, OUT_F * IN_F // 8)).astype(np.int32)
    uu = (rng.standard_normal((W_BIT, OUT_F, RANK)) * 0.05).astype(np.float32)
    vv = (rng.standard_normal((W_BIT, RANK, IN_F)) * 0.05).astype(np.float32)
    out = kernel(x=x, qweight=qw, u=uu, vt=vv)
    print(out.shape, out.dtype)
